# revision 20
# baseline (speedup 1.0000x reference)
"""Trainium2 Bass kernel for nn_Attention_st_2010044694918.

Reference computation (per sample b of B=256):
    q = x[b, :64]                 # [64, 768]
    k = v = x[b, 64:]             # [256, 768]
    S = q @ k.T * 64**-0.5        # [64, 256]
    P = softmax(S, axis=-1)
    out = P @ v                   # [64, 768]
    s = out.T.reshape(64, 768)    # channel-major scramble
    y = s @ proj_w.T + proj_b     # [64, 768]
    result[b] = concat([y, k])    # [320, 768]

Device strategy (pure data parallel, 32 samples / core on 8 cores):
  - S is computed TRANSPOSED (S^T [keys, q]) so no PE transposes or P^T
    copies are needed: lhsT = k^T chunks (128-col weight loads), mov =
    q^T (64 cols), 12 small matmuls, M=128 full array width.
  - softmax has no max subtraction (exps stored in bf16; range is ample)
    and the row sums come from a ones column appended to the PV moving
    operand, so no partition-dim reduction is ever needed.
  - PV is COLUMN-TILED: a pair of samples runs concurrently on the two
    halves of the PE array (sample A -> psum rows 0:64 / tile col 0,
    sample B -> rows 64:128 / tile col 64), with the accumulation groups
    interleaved (verified on HW: has_written clears are per-partition).
  - k is shipped twice (transposed inside xtb for S, natural in knb for
    PV); the knb copy is fp8e4m3 to cut DMA bytes (error budget allows).
  - the scramble is folded into the final matmul: with OUT2 = [out ; out
    shifted left one column], row-pair r=(2c, 2c+1) of the scramble is
    the strided view OUT2[:, 2c::12][:, :64], and y = sum_c of 6
    accumulating matmuls against 128-row slabs of proj_w.T.
  - input DMAs are batched 2 samples per transfer (finer granularity
    measured faster than larger transfers: the pipeline is rate-matched
    with HBM, so arrival granularity dominates transfer efficiency) and
    issued ahead of the later-needed proj_w/bias loads; y stores are
    deferred to a flush at the drain end so they never steal HBM
    bandwidth from the saturated input stream.
"""

import numpy as np
import ml_dtypes

import concourse.bass as bass
import concourse.tile as tile
from concourse import bacc
from concourse import mybir
from concourse.bass_utils import run_bass_kernel_spmd

B, N, C = 256, 320, 768
LZ = 64          # query tokens
LK = N - LZ      # key tokens (256)
NCORES = 8
BS = B // NCORES  # samples per core
SCALE = (C // 12) ** -0.5  # head_dim**-0.5 = 0.125

SB = 2            # samples per input DMA batch
KW = 2 * (C + 1)  # knb per-sample free size: 2 kc chunks x (768 + ones)
KSB = 2           # samples per kn DMA batch
XW = 6 * N        # xtb per-sample free size

F32 = mybir.dt.float32
F16 = mybir.dt.float16
BF16 = mybir.dt.bfloat16
F8 = mybir.dt.float8e4

KN_DT = F8        # dtype of the PV moving operand (k natural + ones col)
EXPS_DT = BF16    # softmax weights (range needs no max subtraction)


def build_nc(bs: int = BS):
    assert bs % 4 == 0
    nb = bs // SB
    nc = bacc.Bacc("TRN2", target_bir_lowering=False)
    xt_d = nc.dram_tensor("xtb", [nb, 128, SB * XW], F16, kind="ExternalInput")
    kn_d = nc.dram_tensor("knb", [bs // KSB, 128, KSB * KW], KN_DT, kind="ExternalInput")
    pwt_d = nc.dram_tensor("pwtb", [128, 6 * C], F16, kind="ExternalInput")
    b64_d = nc.dram_tensor("bias64", [128, C], F16, kind="ExternalInput")
    y_d = nc.dram_tensor("y", [bs * LZ, C], F16, kind="ExternalOutput")

    with tile.TileContext(nc) as tc:
        with (
            tc.tile_pool(name="consts", bufs=1) as consts,
            tc.tile_pool(name="xt", bufs=10) as xt_pool,
            tc.tile_pool(name="kn", bufs=10) as kn_pool,
            tc.tile_pool(name="exps", bufs=4) as exps_pool,
            tc.tile_pool(name="recip", bufs=4) as recip_pool,
            tc.tile_pool(name="out2", bufs=3) as out2_pool,
            tc.tile_pool(name="ysb", bufs=8) as y_pool,
            tc.tile_pool(name="ps_st", bufs=2, space="PSUM") as psum_st,
            tc.tile_pool(name="ps_o", bufs=2, space="PSUM") as psum_o,
            tc.tile_pool(name="ps_y", bufs=1, space="PSUM") as psum_y,
        ):
            pwt_t = consts.tile([128, 6 * C], F16)
            b64_t = consts.tile([128, C], F16)

            st = [dict() for _ in range(bs)]   # per-sample tiles
            flush_list = []
            pr = [dict() for _ in range(bs // 2)]  # per-pair tiles
            batch = [dict() for _ in range(nb)]
            knbatch = [None] * (bs // KSB)

            def stage_load_xt(b):
                if b % SB:
                    return
                bb = b // SB
                xt_t = xt_pool.tile([128, SB * XW], F16, tag="xt")
                nc.sync.dma_start(xt_t[:], xt_d[bb])
                batch[bb]["xt"] = xt_t

            def stage_load_kn(b):
                # trails the xt load so the pair's S^T can start first;
                # coarser batches than xt for better transfer efficiency
                if b % KSB:
                    return
                bb = b // KSB
                kn_t = kn_pool.tile([128, KSB * KW], KN_DT, tag="kn")
                nc.sync.dma_start(kn_t[:], kn_d[bb])
                knbatch[bb] = kn_t

            def stage_consts(b):
                # emitted after the first input batches so the input DMAs
                # win the queue race; these are first needed at proj time
                if b != 0:
                    return
                nc.scalar.dma_start(pwt_t[:], pwt_d[:])
                nc.scalar.dma_start(b64_t[:], b64_d[:])

            def stage_s(b):
                # S^T[key, q] = sum_c k[key,c] q[q,c]: contraction over c in
                # 6 chunks of 128; both key chunks share one psum tile and
                # one accumulation group (per-element has_written handles
                # the first-write-overwrite in each half).
                xt_t = batch[b // SB]["xt"]
                x0 = (b % SB) * XW
                ps_st = psum_st.tile([128, 512], F32, tag="st")  # bank-padded
                for kc in range(2):
                    for cc in range(6):
                        nc.tensor.matmul(
                            ps_st[:, kc * LZ : (kc + 1) * LZ],
                            xt_t[:, x0 + cc * N + LZ + kc * 128 : x0 + cc * N + LZ + (kc + 1) * 128],
                            xt_t[:, x0 + cc * N : x0 + cc * N + LZ],
                            start=(kc == 0 and cc == 0),
                            stop=(kc == 1 and cc == 5),
                        )
                st[b]["ps_st"] = ps_st

            def stage_exp(b):
                # exps[key, q] = exp(S^T); bf16 range handles the unshifted
                # exponent, and normalization cancels it exactly.
                ps_st = st[b].pop("ps_st")
                exps = exps_pool.tile([128, 2 * LZ], EXPS_DT, tag="exps")
                nc.scalar.activation(
                    exps[:], ps_st[:, 0 : 2 * LZ], mybir.ActivationFunctionType.Exp
                )
                st[b]["exps"] = exps

            def stage_av(b):
                # out_unnorm[q, c] = sum_key exps[key,q] k[key,c] for a PAIR
                # of samples column-tiled onto the two array halves; the
                # moving operands carry a trailing ones column so column C
                # holds the softmax row sums. A/B matmuls are interleaved so
                # the halves stream concurrently (separate XBUSes).
                if b % 2 == 0:
                    return
                eA = st[b - 1].pop("exps")
                eB = st[b].pop("exps")
                p = b // 2
                ps_o = psum_o.tile([128, 1024], F32, tag="o")  # bank-padded
                for kc in range(2):
                    for g, ex in ((0, eA), (1, eB)):
                        bs_i = b - 1 + g
                        k0 = (bs_i % KSB) * KW
                        kn_t = knbatch[bs_i // KSB]
                        for h0, h1 in ((0, 512), (512, C + 1)):
                            nc.tensor.matmul(
                                ps_o[g * LZ : (g + 1) * LZ, h0:h1],
                                ex[:, kc * LZ : (kc + 1) * LZ],
                                kn_t[:, k0 + kc * (C + 1) + h0 : k0 + kc * (C + 1) + h1],
                                start=(kc == 0),
                                stop=(kc == 1),
                                # A/B groups interleave in one bank; HW
                                # has_written is per-partition (probed), but
                                # the sim group check is bank-granular
                                skip_group_check=True,
                            )
                pr[p]["ps_o"] = ps_o

            def stage_recip(b):
                if b % 2 == 0:
                    return
                p = b // 2
                recip = recip_pool.tile([128, 1], F32, tag="recip")
                nc.vector.reciprocal(recip[:], pr[p]["ps_o"][:, C : C + 1])
                pr[p]["recip"] = recip

            def stage_norm(b):
                # OUT2 = [out (normalized) ; out shifted left one column];
                # two samples side by side in the free dim of one pair tile
                if b % 2 == 0:
                    return
                p = b // 2
                ps_o = pr[p].pop("ps_o")
                recip = pr[p].pop("recip")
                out2 = out2_pool.tile([128, 2 * C], F16, tag="out2")
                pr[p]["out2"] = out2
                for g in (0, 1):
                    rows = slice(g * LZ, (g + 1) * LZ)
                    co = g * C
                    nc.vector.tensor_scalar_mul(
                        out2[0:LZ, co : co + C], ps_o[rows, 0:C], recip[rows]
                    )
                    # shifted half normalized straight from PSUM on ACT
                    # (parallel with the DVE op above, both read ps_o)
                    nc.scalar.activation(
                        out2[LZ:128, co : co + C - 1],
                        ps_o[rows, 1:C],
                        mybir.ActivationFunctionType.Copy,
                        scale=recip[rows],
                    )

            def stage_proj(b):
                # y = scramble(out) @ proj_w.T for a PAIR of samples: the
                # weight slabs are shared, so sample b-1 fills array columns
                # 0:64 and sample b columns 64:128 (M=128 per matmul)
                if b % 2 == 0:
                    return
                p = b // 2
                out2 = pr[p].pop("out2")
                ps_y = psum_y.tile([128, 1024], F32, tag="ps_y")  # bank-padded
                o2r = out2[:].rearrange("p (g i r) -> p r g i", r=12, g=2)
                for cc2 in range(6):
                    for h0, h1 in ((0, 512), (512, C)):
                        nc.tensor.matmul(
                            ps_y[:, h0:h1],
                            o2r[:, 2 * cc2],
                            pwt_t[:, cc2 * C + h0 : cc2 * C + h1],
                            start=(cc2 == 0),
                            stop=(cc2 == 5),
                        )
                pr[p]["ps_y"] = ps_y

            def stage_y(b):
                # bias add during PSUM eviction; accumulate 2 pairs (4
                # samples) in one SBUF tile, then one 0.4 MB store.
                if b % 2 == 0:
                    return
                p = b // 2
                ps_y = pr[p].pop("ps_y")
                g = (b // 4) * 4   # first sample of the 4-sample store group
                pp = (b // 2) % 2  # pair index within the group
                if pp == 0:
                    ysb = y_pool.tile([128, 2 * C], F16, tag="ysb")
                    st[g]["ysb"] = ysb
                else:
                    ysb = st[g]["ysb"]
                nc.vector.tensor_add(ysb[:, pp * C : (pp + 1) * C], ps_y[:, 0:C], b64_t[:])
                if pp == 1:
                    st[g].pop("ysb")
                    flush_list.append((g, ysb))

            def stage_flush(b):
                # all y stores issue at the drain end, after the input DMA
                # stream has finished, so they never steal HBM bandwidth
                # from the (saturated) input loads
                if b != bs - 1:
                    return
                for g, ysb in flush_list:
                    dst = y_d[g * LZ : (g + 4) * LZ, :].rearrange(
                        "(pr2 p) c -> p pr2 c", pr2=2
                    )
                    nc.scalar.dma_start(
                        dst, ysb[:].rearrange("p (pr2 c) -> p pr2 c", pr2=2)
                    )

            stages = [
                (stage_load_xt, 0),
                (stage_load_kn, 1),
                (stage_s, 4),
                (stage_exp, 6),
                (stage_consts, 7),
                (stage_av, 7),
                (stage_recip, 8),
                (stage_norm, 9),
                (stage_proj, 11),
                (stage_y, 12),
                (stage_flush, 13),
            ]
            max_skew = max(sk for _, sk in stages)
            for i in range(bs + max_skew):
                for fn, sk in stages:
                    b = i - sk
                    if 0 <= b < bs:
                        fn(b)

    nc.compile()
    return nc


_NC_CACHE = {}


def _get_nc(bs: int = BS):
    if bs not in _NC_CACHE:
        _NC_CACHE[bs] = build_nc(bs)
    return _NC_CACHE[bs]


def _host_prep(x, proj_w, proj_b):
    """Pre-block inputs into the exact SBUF layouts (contiguous DMAs)."""
    x = np.asarray(x, dtype=np.float32)
    proj_w = np.asarray(proj_w, dtype=np.float32)
    proj_b = np.asarray(proj_b, dtype=np.float32)

    f16 = np.float16
    kn_np = mybir.dt.np(KN_DT)
    # xtb[b, p, cc*N + t] = x[b, t, cc*128 + p]; the softmax scale is folded
    # into the query columns (t < LZ) so S arrives pre-scaled
    xtb = x.reshape(B, N, 6, 128).transpose(0, 3, 2, 1).reshape(B, 128, 6 * N)
    xtb = np.ascontiguousarray(xtb, dtype=np.float32).reshape(B, 128, 6, N)
    xtb[:, :, :, :LZ] *= SCALE
    # batch SB samples side by side in the free dim
    xtb = xtb.reshape(B // SB, SB, 128, XW).transpose(0, 2, 1, 3)
    xtb = np.ascontiguousarray(xtb, dtype=f16).reshape(B // SB, 128, SB * XW)
    # knb[b, p, kc*(C+1) + c] = x[b, LZ + kc*128 + p, c], col C = ones
    knb = np.ones((B, 128, 2, C + 1), dtype=np.float32)
    knb[:, :, :, :C] = x[:, LZ:, :].reshape(B, 2, 128, C).transpose(0, 2, 1, 3)
    knb = knb.reshape(B // KSB, KSB, 128, KW).transpose(0, 2, 1, 3)
    knb = np.ascontiguousarray(knb.astype(kn_np)).reshape(B // KSB, 128, KSB * KW)
    # pwtb[p, cc*C + m] = proj_w.T[cc*128 + p, m] = proj_w[m, cc*128 + p]
    pwtb = np.ascontiguousarray(
        proj_w.T.reshape(6, 128, C).transpose(1, 0, 2).reshape(128, 6 * C),
        dtype=f16,
    )
    b64 = np.ascontiguousarray(np.broadcast_to(proj_b, (128, C)), dtype=f16)
    return x, xtb, knb, pwtb, b64


def _run(x, proj_w, proj_b, **spmd_kwargs):
    x, xtb, knb, pwtb, b64 = _host_prep(x, proj_w, proj_b)

    nc = _get_nc()
    nbc = BS // SB  # batches per core
    in_maps = [
        {
            "xtb": xtb[i * nbc : (i + 1) * nbc],
            "knb": knb[i * (BS // KSB) : (i + 1) * (BS // KSB)],
            "pwtb": pwtb,
            "bias64": b64,
        }
        for i in range(NCORES)
    ]
    res = run_bass_kernel_spmd(
        nc, in_maps, core_ids=list(range(NCORES)), **spmd_kwargs
    )

    out = np.empty((B, N, C), dtype=np.float32)
    out[:, LZ:, :] = x[:, LZ:, :]
    for i in range(NCORES):
        out[i * BS : (i + 1) * BS, :LZ, :] = res.results[i]["y"].reshape(BS, LZ, C)
    return out, res


def kernel(x, proj_w, proj_b):
    out, _ = _run(x, proj_w, proj_b)
    return out


# revision 22
# speedup vs baseline: 1.0079x; 1.0079x over previous
"""Trainium2 Bass kernel for nn_Attention_st_2010044694918.

Reference computation (per sample b of B=256):
    q = x[b, :64]                 # [64, 768]
    k = v = x[b, 64:]             # [256, 768]
    S = q @ k.T * 64**-0.5        # [64, 256]
    P = softmax(S, axis=-1)
    out = P @ v                   # [64, 768]
    s = out.T.reshape(64, 768)    # channel-major scramble
    y = s @ proj_w.T + proj_b     # [64, 768]
    result[b] = concat([y, k])    # [320, 768]

Device strategy (pure data parallel, 32 samples / core on 8 cores):
  - S is computed TRANSPOSED (S^T [keys, q]) so no PE transposes or P^T
    copies are needed: lhsT = k^T chunks (128-col weight loads), mov =
    q^T (64 cols), 12 small matmuls, M=128 full array width.
  - softmax has no max subtraction (exps stored in bf16; range is ample)
    and the row sums come from a ones column appended to the PV moving
    operand, so no partition-dim reduction is ever needed.
  - PV is COLUMN-TILED: a pair of samples runs concurrently on the two
    halves of the PE array (sample A -> psum rows 0:64 / tile col 0,
    sample B -> rows 64:128 / tile col 64), with the accumulation groups
    interleaved (verified on HW: has_written clears are per-partition).
  - k is shipped twice (transposed inside xtb for S, natural in knb for
    PV); the knb copy is fp8e4m3 to cut DMA bytes (error budget allows).
  - the scramble is folded into the final matmul: with OUT2 = [out ; out
    shifted left one column], row-pair r=(2c, 2c+1) of the scramble is
    the strided view OUT2[:, 2c::12][:, :64], and y = sum_c of 6
    accumulating matmuls against 128-row slabs of proj_w.T.
  - input DMAs are batched 2 samples per transfer (finer granularity
    measured faster than larger transfers: the pipeline is rate-matched
    with HBM, so arrival granularity dominates transfer efficiency) and
    issued ahead of the later-needed proj_w/bias loads; y stores are
    deferred to a flush at the drain end so they never steal HBM
    bandwidth from the saturated input stream.
"""

import numpy as np
import ml_dtypes

import concourse.bass as bass
import concourse.tile as tile
from concourse import bacc
from concourse import mybir
from concourse.bass_utils import run_bass_kernel_spmd

B, N, C = 256, 320, 768
LZ = 64          # query tokens
LK = N - LZ      # key tokens (256)
NCORES = 8
BS = B // NCORES  # samples per core
SCALE = (C // 12) ** -0.5  # head_dim**-0.5 = 0.125

SB = 2            # samples per input DMA batch
KW = 2 * (C + 1)  # knb per-sample free size: 2 kc chunks x (768 + ones)
KSB = 2           # samples per kn DMA batch
XW = 6 * N        # xtb per-sample free size

F32 = mybir.dt.float32
F16 = mybir.dt.float16
BF16 = mybir.dt.bfloat16
F8 = mybir.dt.float8e4

KN_DT = F8        # dtype of the PV moving operand (k natural + ones col)
EXPS_DT = BF16    # softmax weights (range needs no max subtraction)


def build_nc(bs: int = BS):
    assert bs % 4 == 0
    nb = bs // SB
    nc = bacc.Bacc("TRN2", target_bir_lowering=False)
    xt_d = nc.dram_tensor("xtb", [nb, 128, SB * XW], F16, kind="ExternalInput")
    kn_d = nc.dram_tensor("knb", [bs // KSB, 128, KSB * KW], KN_DT, kind="ExternalInput")
    cst_d = nc.dram_tensor("cstb", [128, 7 * C], F16, kind="ExternalInput")
    y_d = nc.dram_tensor("y", [bs * LZ, C], F16, kind="ExternalOutput")

    with tile.TileContext(nc) as tc:
        with (
            tc.tile_pool(name="consts", bufs=1) as consts,
            tc.tile_pool(name="xt", bufs=10) as xt_pool,
            tc.tile_pool(name="kn", bufs=10) as kn_pool,
            tc.tile_pool(name="exps", bufs=4) as exps_pool,
            tc.tile_pool(name="recip", bufs=4) as recip_pool,
            tc.tile_pool(name="out2", bufs=3) as out2_pool,
            tc.tile_pool(name="ysb", bufs=8) as y_pool,
            tc.tile_pool(name="ps_st", bufs=2, space="PSUM") as psum_st,
            tc.tile_pool(name="ps_o", bufs=2, space="PSUM") as psum_o,
            tc.tile_pool(name="ps_y", bufs=1, space="PSUM") as psum_y,
        ):
            cst_t = consts.tile([128, 7 * C], F16)
            pwt_t = cst_t[:, 0 : 6 * C]
            b64_t = cst_t[:, 6 * C : 7 * C]

            st = [dict() for _ in range(bs)]   # per-sample tiles
            flush_list = []
            pr = [dict() for _ in range(bs // 2)]  # per-pair tiles
            batch = [dict() for _ in range(nb)]
            knbatch = [None] * (bs // KSB)

            def stage_load_xt(b):
                if b % SB:
                    return
                bb = b // SB
                xt_t = xt_pool.tile([128, SB * XW], F16, tag="xt")
                nc.sync.dma_start(xt_t[:], xt_d[bb])
                batch[bb]["xt"] = xt_t

            def stage_load_kn(b):
                # trails the xt load so the pair's S^T can start first;
                # coarser batches than xt for better transfer efficiency
                if b % KSB:
                    return
                bb = b // KSB
                kn_t = kn_pool.tile([128, KSB * KW], KN_DT, tag="kn")
                nc.sync.dma_start(kn_t[:], kn_d[bb])
                knbatch[bb] = kn_t

            def stage_consts(b):
                # one merged proj_w+bias transfer, on the SYNC queue in
                # FIFO position after batches 0-1: stays out of the
                # critical ramp window, lands just before first proj use
                if b != 0:
                    return
                nc.sync.dma_start(cst_t[:], cst_d[:])

            def stage_s(b):
                # S^T[key, q] = sum_c k[key,c] q[q,c]: contraction over c in
                # 6 chunks of 128; both key chunks share one psum tile and
                # one accumulation group (per-element has_written handles
                # the first-write-overwrite in each half).
                xt_t = batch[b // SB]["xt"]
                x0 = (b % SB) * XW
                ps_st = psum_st.tile([128, 512], F32, tag="st")  # bank-padded
                for kc in range(2):
                    for cc in range(6):
                        nc.tensor.matmul(
                            ps_st[:, kc * LZ : (kc + 1) * LZ],
                            xt_t[:, x0 + cc * N + LZ + kc * 128 : x0 + cc * N + LZ + (kc + 1) * 128],
                            xt_t[:, x0 + cc * N : x0 + cc * N + LZ],
                            start=(kc == 0 and cc == 0),
                            stop=(kc == 1 and cc == 5),
                        )
                st[b]["ps_st"] = ps_st

            def stage_exp(b):
                # exps[key, q] = exp(S^T); bf16 range handles the unshifted
                # exponent, and normalization cancels it exactly.
                ps_st = st[b].pop("ps_st")
                exps = exps_pool.tile([128, 2 * LZ], EXPS_DT, tag="exps")
                nc.scalar.activation(
                    exps[:], ps_st[:, 0 : 2 * LZ], mybir.ActivationFunctionType.Exp
                )
                st[b]["exps"] = exps

            def stage_av(b):
                # out_unnorm[q, c] = sum_key exps[key,q] k[key,c] for a PAIR
                # of samples column-tiled onto the two array halves; the
                # moving operands carry a trailing ones column so column C
                # holds the softmax row sums. A/B matmuls are interleaved so
                # the halves stream concurrently (separate XBUSes).
                if b % 2 == 0:
                    return
                eA = st[b - 1].pop("exps")
                eB = st[b].pop("exps")
                p = b // 2
                ps_o = psum_o.tile([128, 1024], F32, tag="o")  # bank-padded
                for kc in range(2):
                    for g, ex in ((0, eA), (1, eB)):
                        bs_i = b - 1 + g
                        k0 = (bs_i % KSB) * KW
                        kn_t = knbatch[bs_i // KSB]
                        for h0, h1 in ((0, 512), (512, C + 1)):
                            nc.tensor.matmul(
                                ps_o[g * LZ : (g + 1) * LZ, h0:h1],
                                ex[:, kc * LZ : (kc + 1) * LZ],
                                kn_t[:, k0 + kc * (C + 1) + h0 : k0 + kc * (C + 1) + h1],
                                start=(kc == 0),
                                stop=(kc == 1),
                                # A/B groups interleave in one bank; HW
                                # has_written is per-partition (probed), but
                                # the sim group check is bank-granular
                                skip_group_check=True,
                            )
                pr[p]["ps_o"] = ps_o

            def stage_recip(b):
                if b % 2 == 0:
                    return
                p = b // 2
                recip = recip_pool.tile([128, 1], F32, tag="recip")
                nc.vector.reciprocal(recip[:], pr[p]["ps_o"][:, C : C + 1])
                pr[p]["recip"] = recip

            def stage_norm(b):
                # OUT2 = [out (normalized) ; out shifted left one column];
                # two samples side by side in the free dim of one pair tile
                if b % 2 == 0:
                    return
                p = b // 2
                ps_o = pr[p].pop("ps_o")
                recip = pr[p].pop("recip")
                out2 = out2_pool.tile([128, 2 * C], F16, tag="out2")
                pr[p]["out2"] = out2
                for g in (0, 1):
                    rows = slice(g * LZ, (g + 1) * LZ)
                    co = g * C
                    nc.vector.tensor_scalar_mul(
                        out2[0:LZ, co : co + C], ps_o[rows, 0:C], recip[rows]
                    )
                    # shifted half normalized straight from PSUM on ACT
                    # (parallel with the DVE op above, both read ps_o)
                    nc.scalar.activation(
                        out2[LZ:128, co : co + C - 1],
                        ps_o[rows, 1:C],
                        mybir.ActivationFunctionType.Copy,
                        scale=recip[rows],
                    )

            def stage_proj(b):
                # y = scramble(out) @ proj_w.T for a PAIR of samples: the
                # weight slabs are shared, so sample b-1 fills array columns
                # 0:64 and sample b columns 64:128 (M=128 per matmul)
                if b % 2 == 0:
                    return
                p = b // 2
                out2 = pr[p].pop("out2")
                ps_y = psum_y.tile([128, 1024], F32, tag="ps_y")  # bank-padded
                o2r = out2[:].rearrange("p (g i r) -> p r g i", r=12, g=2)
                for cc2 in range(6):
                    for h0, h1 in ((0, 512), (512, C)):
                        nc.tensor.matmul(
                            ps_y[:, h0:h1],
                            o2r[:, 2 * cc2],
                            pwt_t[:, cc2 * C + h0 : cc2 * C + h1],
                            start=(cc2 == 0),
                            stop=(cc2 == 5),
                        )
                pr[p]["ps_y"] = ps_y

            def stage_y(b):
                # bias add during PSUM eviction; accumulate 2 pairs (4
                # samples) in one SBUF tile, then one 0.4 MB store.
                if b % 2 == 0:
                    return
                p = b // 2
                ps_y = pr[p].pop("ps_y")
                g = (b // 4) * 4   # first sample of the 4-sample store group
                pp = (b // 2) % 2  # pair index within the group
                if pp == 0:
                    ysb = y_pool.tile([128, 2 * C], F16, tag="ysb")
                    st[g]["ysb"] = ysb
                else:
                    ysb = st[g]["ysb"]
                nc.vector.tensor_add(ysb[:, pp * C : (pp + 1) * C], ps_y[:, 0:C], b64_t)
                if pp == 1:
                    st[g].pop("ysb")
                    flush_list.append((g, ysb))

            def stage_flush(b):
                # all y stores issue at the drain end, after the input DMA
                # stream has finished, so they never steal HBM bandwidth
                # from the (saturated) input loads
                if b != bs - 1:
                    return
                for g, ysb in flush_list:
                    dst = y_d[g * LZ : (g + 4) * LZ, :].rearrange(
                        "(pr2 p) c -> p pr2 c", pr2=2
                    )
                    nc.scalar.dma_start(
                        dst, ysb[:].rearrange("p (pr2 c) -> p pr2 c", pr2=2)
                    )

            stages = [
                (stage_load_xt, 0),
                (stage_load_kn, 1),
                (stage_s, 4),
                (stage_exp, 6),
                (stage_consts, 7),
                (stage_av, 7),
                (stage_recip, 8),
                (stage_norm, 9),
                (stage_proj, 11),
                (stage_y, 12),
                (stage_flush, 13),
            ]
            max_skew = max(sk for _, sk in stages)
            for i in range(bs + max_skew):
                for fn, sk in stages:
                    b = i - sk
                    if 0 <= b < bs:
                        fn(b)

    nc.compile()
    return nc


_NC_CACHE = {}


def _get_nc(bs: int = BS):
    if bs not in _NC_CACHE:
        _NC_CACHE[bs] = build_nc(bs)
    return _NC_CACHE[bs]


def _host_prep(x, proj_w, proj_b):
    """Pre-block inputs into the exact SBUF layouts (contiguous DMAs)."""
    x = np.asarray(x, dtype=np.float32)
    proj_w = np.asarray(proj_w, dtype=np.float32)
    proj_b = np.asarray(proj_b, dtype=np.float32)

    f16 = np.float16
    kn_np = mybir.dt.np(KN_DT)
    # xtb[b, p, cc*N + t] = x[b, t, cc*128 + p]; the softmax scale is folded
    # into the query columns (t < LZ) so S arrives pre-scaled
    xtb = x.reshape(B, N, 6, 128).transpose(0, 3, 2, 1).reshape(B, 128, 6 * N)
    xtb = np.ascontiguousarray(xtb, dtype=np.float32).reshape(B, 128, 6, N)
    xtb[:, :, :, :LZ] *= SCALE
    # batch SB samples side by side in the free dim
    xtb = xtb.reshape(B // SB, SB, 128, XW).transpose(0, 2, 1, 3)
    xtb = np.ascontiguousarray(xtb, dtype=f16).reshape(B // SB, 128, SB * XW)
    # knb[b, p, kc*(C+1) + c] = x[b, LZ + kc*128 + p, c], col C = ones
    knb = np.ones((B, 128, 2, C + 1), dtype=np.float32)
    knb[:, :, :, :C] = x[:, LZ:, :].reshape(B, 2, 128, C).transpose(0, 2, 1, 3)
    knb = knb.reshape(B // KSB, KSB, 128, KW).transpose(0, 2, 1, 3)
    knb = np.ascontiguousarray(knb.astype(kn_np)).reshape(B // KSB, 128, KSB * KW)
    # pwtb[p, cc*C + m] = proj_w.T[cc*128 + p, m] = proj_w[m, cc*128 + p]
    pwtb = np.ascontiguousarray(
        proj_w.T.reshape(6, 128, C).transpose(1, 0, 2).reshape(128, 6 * C),
        dtype=f16,
    )
    b64 = np.broadcast_to(proj_b, (128, C))
    cst = np.ascontiguousarray(np.concatenate([pwtb, b64], axis=1), dtype=f16)
    return x, xtb, knb, cst


def _run(x, proj_w, proj_b, **spmd_kwargs):
    x, xtb, knb, cst = _host_prep(x, proj_w, proj_b)

    nc = _get_nc()
    nbc = BS // SB  # batches per core
    in_maps = [
        {
            "xtb": xtb[i * nbc : (i + 1) * nbc],
            "knb": knb[i * (BS // KSB) : (i + 1) * (BS // KSB)],
            "cstb": cst,
        }
        for i in range(NCORES)
    ]
    res = run_bass_kernel_spmd(
        nc, in_maps, core_ids=list(range(NCORES)), **spmd_kwargs
    )

    out = np.empty((B, N, C), dtype=np.float32)
    out[:, LZ:, :] = x[:, LZ:, :]
    for i in range(NCORES):
        out[i * BS : (i + 1) * BS, :LZ, :] = res.results[i]["y"].reshape(BS, LZ, C)
    return out, res


def kernel(x, proj_w, proj_b):
    out, _ = _run(x, proj_w, proj_b)
    return out
